# revision 4
# baseline (speedup 1.0000x reference)
"""ChiENN layer Trainium2 kernel (8-core data-parallel SPMD), v2.

Math (valid for inputs with no -1 padding in circle_index, which the input
spec guarantees: randint in [0, N)):

    num_neighbors == 4 for every node, all masks all-true, and the K=3
    window sum never crosses a node's circle row. Hence per node n:

      h_c   = sum_{i<3} x[ci[n, c+i]] @ Wk[i] + B3          (c = 0..3)
      msg_c = elu(h_c) @ Wf + bf
      agg   = sum_c msg_c + x[n]@Ws + bs + x[pni[n]]@Wp + bp
      out   = elu(agg)

    with B3 = bk[0]+bk[1]+bk[2].  Using elu(z) = relu(z) + exp(min(z,0)) - 1
    the per-window "-1" collapses into a host-side constant:
      sum_c elu(h_c) @ Wf = (sum_c relu(.) + sum_c exp(min(.,0))) @ Wf - 4*colsum(Wf)
    so cb = 4*bf + bs + bp - 4*colsum(Wf) is applied once pre-final-elu, and
    the final elu uses  out = max(z-1, -1) + exp(min(z, 0)).

Per-core layout: nodes sharded contiguously, 12800 per core (N padded to
102400), processed in 25 blocks of 512 nodes (4 subtiles of 128).  All
gathers (6 circle + 1 parallel per node) are ONE merged indirect DMA per
block (3584 descriptors of 256B; the ~1us fixed SWDGE overhead is paid once
per block instead of 28x).  Everything on the matmul path is bf16; PSUM
accumulation stays fp32.  Gathered tiles are PE-transposed (bf16) into
[D, nodes] layout; a one-block software-pipeline skew keeps the PE busy
while the scalar/vector engines produce the elu pieces.  The output is
produced transposed ([D, nodes]) in bf16 and un-transposed on the host.
"""

import numpy as np
import ml_dtypes

import concourse.bass as bass
import concourse.mybir as mybir
import concourse.tile as tile
from concourse import bacc
from concourse.bass import IndirectOffsetOnAxis
from concourse.bass_utils import run_bass_kernel_spmd
from concourse.masks import make_identity

N, D, C, K = 100000, 128, 6, 3
NCORES = 8
BLK = 512                 # nodes per block
SUB = BLK // 128          # subtiles per block
NB = 25                   # blocks per core
NS = BLK * NB             # nodes per core (12800)
NPAD = NS * NCORES        # padded node count (102400)
NSLOT = C + 1             # 6 circle + 1 parallel gather (self is a dense load)
NW = C - K + 1            # 4 windows

F32 = mybir.dt.float32
BF16 = mybir.dt.bfloat16
I32 = mybir.dt.int32
NPBF16 = ml_dtypes.bfloat16

AF = mybir.ActivationFunctionType
OP = mybir.AluOpType


def build_module(nb: int = NB):
    """Build + compile the per-core Bass program (same program on all cores)."""
    nc = bacc.Bacc("TRN2", target_bir_lowering=False, debug=False,
                   num_devices=NCORES, num_swdge_queues=2,
                   dynamic_dma_scratch_size=65536)

    ns = nb * BLK
    xg = nc.dram_tensor("xg", [N, D], BF16, kind="ExternalInput").ap()
    xts = nc.dram_tensor("xts", [D, ns], BF16, kind="ExternalInput").ap()
    idxg = nc.dram_tensor("idxg", [nb, 128, NSLOT * SUB], I32,
                          kind="ExternalInput").ap()
    wdram = {
        name: nc.dram_tensor(name, [D, D], BF16, kind="ExternalInput").ap()
        for name in ("wk0", "wk1", "wk2", "wf", "ws", "wp")
    }
    b3d = nc.dram_tensor("b3", [D, 1], F32, kind="ExternalInput").ap()
    cbd = nc.dram_tensor("cb", [D, 1], F32, kind="ExternalInput").ap()
    cb1d = nc.dram_tensor("cb1", [D, 1], F32, kind="ExternalInput").ap()
    yt = nc.dram_tensor("yt", [D, ns], BF16, kind="ExternalOutput").ap()

    with tile.TileContext(nc) as tc:
        with (
            tc.tile_pool(name="const", bufs=1) as constp,
            tc.tile_pool(name="gp", bufs=3) as gp,
            tc.tile_pool(name="gtp", bufs=2) as gtp,
            tc.tile_pool(name="vep", bufs=2) as vep,
            tc.tile_pool(name="finp", bufs=2) as finp,
            tc.tile_pool(name="outp", bufs=2) as outp,
            tc.tile_pool(name="idxp", bufs=6) as idxp,
            tc.tile_pool(name="xtp", bufs=4) as xtp,
            tc.tile_pool(name="pst", bufs=2, space="PSUM") as pst,
            tc.tile_pool(name="psh", bufs=1, space="PSUM") as psh,
            tc.tile_pool(name="psm", bufs=2, space="PSUM") as psm,
        ):
            # ---- constants ----
            wsb = {}
            for name in ("wk0", "wk1", "wk2", "wf", "ws", "wp"):
                w_t = constp.tile([D, D], BF16, name=f"{name}_s")
                nc.sync.dma_start(out=w_t[:], in_=wdram[name][:])
                wsb[name] = w_t
            b3_t = constp.tile([D, 1], F32, name="b3_s")
            nc.sync.dma_start(out=b3_t[:], in_=b3d[:])
            cb_t = constp.tile([D, 1], F32, name="cb_s")
            nc.sync.dma_start(out=cb_t[:], in_=cbd[:])
            cb1_t = constp.tile([D, 1], F32, name="cb1_s")
            nc.sync.dma_start(out=cb1_t[:], in_=cb1d[:])
            ident = constp.tile([D, D], BF16, name="ident")
            make_identity(nc, ident[:])

            wk_s = [wsb["wk0"], wsb["wk1"], wsb["wk2"]]

            prev = None  # (m, v, e) of the previous block
            for b in range(nb + 1):
                if b < nb:
                    # ---- loads ----
                    idx_t = idxp.tile([128, NSLOT * SUB], I32, name="idx_t")
                    nc.sync.dma_start(out=idx_t[:], in_=idxg[b])
                    xt_r = xtp.tile([D, BLK], BF16, name="xt_r", tag="xt")
                    nc.sync.dma_start(out=xt_r[:],
                                      in_=xts[:, b * BLK:(b + 1) * BLK])

                    # ---- gathers: g[p, s*D:(s+1)*D] = x[idx[p, s]].  The HW
                    # SWDGE consumes ONE index per partition per instruction,
                    # so issue one gather per (slot, subtile).
                    g = gp.tile([128, NSLOT * SUB * D], BF16, name="g")
                    for s in range(NSLOT * SUB):
                        gi = nc.gpsimd.indirect_dma_start(
                            out=g[:, s * D:(s + 1) * D],
                            out_offset=None,
                            in_=xg[:],
                            in_offset=IndirectOffsetOnAxis(
                                ap=idx_t[:, s:s + 1], axis=0),
                        )
                        if s % 2:
                            gi.ins.queue = "qPoolDynamic1"

                    # ---- transpose gathered tiles to [D, nodes], in pairs ----
                    slot_ap = [None] * NSLOT
                    for p in range(4):
                        cnt = 2 if p < 3 else 1
                        ps_t = pst.tile([D, 1024], BF16, name="ps_t", tag="ps")
                        for q in range(cnt):
                            j = 2 * p + q
                            for t in range(SUB):
                                s = j * SUB + t
                                nc.tensor.transpose(
                                    ps_t[:, q * BLK + t * 128:
                                         q * BLK + (t + 1) * 128],
                                    g[:, s * D:(s + 1) * D],
                                    ident[:],
                                )
                        gt_p = gtp.tile([D, 1024], BF16, name=f"gt{p}",
                                        tag=f"gt{p}")
                        if p % 2 == 0:
                            nc.scalar.copy(out=gt_p[:, :cnt * BLK],
                                           in_=ps_t[:, :cnt * BLK])
                        else:
                            nc.vector.tensor_copy(out=gt_p[:, :cnt * BLK],
                                                  in_=ps_t[:, :cnt * BLK])
                        for q in range(cnt):
                            slot_ap[2 * p + q] = gt_p[:, q * BLK:(q + 1) * BLK]

                    # ---- window matmuls: h_c = sum_i Wk[i].T @ gt_{c+i} ----
                    h = psh.tile([D, NW * BLK], F32, name="h", tag="h")
                    for i in range(K):
                        for c in range(NW):
                            nc.tensor.matmul(
                                h[:, c * BLK:(c + 1) * BLK],
                                lhsT=wk_s[i][:],
                                rhs=slot_ap[c + i],
                                start=(i == 0),
                                stop=(i == K - 1),
                            )

                    # ---- aggregate bank: m = self + parallel (+ Wf terms later)
                    m = psm.tile([D, BLK], F32, name="m", tag="m")
                    nc.tensor.matmul(m[:], lhsT=wsb["ws"][:], rhs=xt_r[:],
                                     start=True, stop=False)
                    nc.tensor.matmul(m[:], lhsT=wsb["wp"][:], rhs=slot_ap[C],
                                     start=False, stop=False)

                    # ---- elu pieces over all 4 windows at once ----
                    v = vep.tile([D, NW * BLK], BF16, name="v", tag="v")
                    nc.scalar.activation(v[:], h[:], AF.Relu, bias=b3_t[:, :1])
                    tmin = vep.tile([D, NW * BLK], BF16, name="tmin", tag="t")
                    nc.vector.tensor_scalar(
                        out=tmin[:], in0=h[:], scalar1=b3_t[:, :1],
                        scalar2=0.0, op0=OP.add, op1=OP.min)
                    e = vep.tile([D, NW * BLK], BF16, name="e", tag="e")
                    nc.scalar.activation(e[:], tmin[:], AF.Exp)

                if b >= 1:
                    m_p, v_p, e_p = prev
                    for c in range(NW):
                        nc.tensor.matmul(m_p[:], lhsT=wsb["wf"][:],
                                         rhs=v_p[:, c * BLK:(c + 1) * BLK],
                                         start=False, stop=False)
                        nc.tensor.matmul(m_p[:], lhsT=wsb["wf"][:],
                                         rhs=e_p[:, c * BLK:(c + 1) * BLK],
                                         start=False, stop=(c == NW - 1))
                    # ---- final elu(m + cb) = max(z-1,-1) + exp(min(z,0)) ----
                    v_f = finp.tile([D, BLK], F32, name="v_f", tag="vf")
                    nc.vector.tensor_scalar(
                        out=v_f[:], in0=m_p[:], scalar1=cb1_t[:, :1],
                        scalar2=-1.0, op0=OP.add, op1=OP.max)
                    t_f = finp.tile([D, BLK], BF16, name="t_f", tag="tf")
                    nc.vector.tensor_scalar(
                        out=t_f[:], in0=m_p[:], scalar1=cb_t[:, :1],
                        scalar2=0.0, op0=OP.add, op1=OP.min)
                    e_f = finp.tile([D, BLK], F32, name="e_f", tag="ef")
                    nc.scalar.activation(e_f[:], t_f[:], AF.Exp)
                    o_t = outp.tile([D, BLK], BF16, name="o_t", tag="o")
                    nc.vector.tensor_tensor(out=o_t[:], in0=v_f[:],
                                            in1=e_f[:], op=OP.add)
                    nc.sync.dma_start(out=yt[:, (b - 1) * BLK:b * BLK],
                                      in_=o_t[:])

                if b < nb:
                    prev = (m, v, e)

    nc.compile()
    return nc


def build_in_maps(inputs: dict, nb: int = NB):
    """Shard/arrange FULL inputs into 8 per-core input maps."""
    x = np.ascontiguousarray(np.asarray(inputs["x"], dtype=np.float32))
    ci = np.asarray(inputs["circle_index"], dtype=np.int32)
    pni = np.asarray(inputs["parallel_node_index"], dtype=np.int32)
    Wk = np.asarray(inputs["Wk"], dtype=np.float32)
    bk = np.asarray(inputs["bk"], dtype=np.float32)
    Wf = np.ascontiguousarray(np.asarray(inputs["Wf"], dtype=np.float32))
    bf = np.asarray(inputs["bf"], dtype=np.float32)
    Ws = np.ascontiguousarray(np.asarray(inputs["Ws"], dtype=np.float32))
    bs = np.asarray(inputs["bs"], dtype=np.float32)
    Wp = np.ascontiguousarray(np.asarray(inputs["Wp"], dtype=np.float32))
    bp = np.asarray(inputs["bp"], dtype=np.float32)

    ns = nb * BLK
    npad = ns * NCORES

    idx_all = np.concatenate([ci, pni[:, None]], axis=1)            # [N, 7]
    idx_all = np.clip(idx_all, 0, N - 1)
    idx_pad = np.zeros((npad, NSLOT), np.int32)
    idx_pad[:N] = idx_all[: min(N, npad)]

    x_bf = x.astype(NPBF16)
    xt = np.zeros((D, npad), NPBF16)
    xt[:, :N] = x_bf.T[:, : min(N, npad)]

    wf_bf = Wf.astype(NPBF16)
    b3 = (bk[0] + bk[1] + bk[2]).reshape(D, 1).astype(np.float32)
    cb = (4.0 * bf + bs + bp
          - 4.0 * wf_bf.astype(np.float32).sum(axis=0)
          ).reshape(D, 1).astype(np.float32)
    cb1 = cb - 1.0

    common = {
        "xg": x_bf,
        "wk0": np.ascontiguousarray(Wk[0].astype(NPBF16)),
        "wk1": np.ascontiguousarray(Wk[1].astype(NPBF16)),
        "wk2": np.ascontiguousarray(Wk[2].astype(NPBF16)),
        "wf": np.ascontiguousarray(wf_bf),
        "ws": Ws.astype(NPBF16),
        "wp": Wp.astype(NPBF16),
        "b3": b3, "cb": cb, "cb1": cb1,
    }
    in_maps = []
    for c in range(NCORES):
        sl = idx_pad[c * ns:(c + 1) * ns]                     # [ns, NSLOT]
        # -> [nb, 128(p), NSLOT(j)*SUB(t)] with slot s = j*SUB + t
        idxc = (sl.reshape(nb, SUB, 128, NSLOT)
                  .transpose(0, 2, 3, 1)
                  .reshape(nb, 128, NSLOT * SUB))
        in_maps.append({
            **common,
            "idxg": np.ascontiguousarray(idxc),
            "xts": np.ascontiguousarray(xt[:, c * ns:(c + 1) * ns]),
        })
    return in_maps


def assemble_output(results, nb: int = NB):
    ns = nb * BLK
    out = np.empty((NCORES * ns, D), np.float32)
    for c in range(NCORES):
        out[c * ns:(c + 1) * ns] = results[c]["yt"].T.astype(np.float32)
    return np.ascontiguousarray(out[:N])


_NC_CACHE = {}


def kernel(**inputs) -> np.ndarray:
    if "nc" not in _NC_CACHE:
        _NC_CACHE["nc"] = build_module()
    nc = _NC_CACHE["nc"]
    in_maps = build_in_maps(inputs)
    res = run_bass_kernel_spmd(nc, in_maps, core_ids=list(range(NCORES)))
    return assemble_output(res.results)
